# revision 29
# baseline (speedup 1.0000x reference)
"""GumbelQuantizer forward on 8 Trainium2 NeuronCores.

Strategy (data-parallel over the bs*l token axis, per the sharding hint):
  - 32768 tokens are split into 8 shards of 4096 tokens; each core runs an
    identical Bass/Tile program on its shard. Weights + codebook replicated.
  - Per core:  hT = gelu(W1.T @ xT + b1)   (PE, f32r full-rate matmuls)
               logits = hT.T @ W2          (PE, token-major output)
               z = logits + (gumbels + b2) (DVE add; b2 pre-folded into
                                            gumbels on host)
               idx = argmax(z) per group   (DVE max/max_index)
               out = emb[idx]              (indirect-DMA gather per (t,g))
  - The straight-through estimator's forward value is hard one-hot up to
    ~1.2e-7, so the softmax itself is skipped and the output is the gathered
    codebook row (exact fp32).
  - PE is the bottleneck engine (~140 us of f32r matmul at the sustained
    ~2.2 GHz clock), so the program is software-pipelined two chunks deep:
    body c issues mm1(chunk c) then mm2(chunk c-2).  This hides the gelu
    latency between the two matmuls AND delays the first W2/gumbel use to
    ~25 us, which matters because the startup is chip-HBM-bound (all 8 cores
    burst-load weights at t=0).  Warmup matmuls on a zeroed tile ramp the PE
    p-state during the DMA prologue.  Loads are spread over the DGE queues:
    SP: x + output stores, Act: W1+b1+gumbels, Pool/SWDGE: W2 + first gumbels.
"""

import os
import sys

sys.path.insert(0, "/opt/trn_rl_repo")

import numpy as np

NCORES = 8
BS, L, DIM = 16, 2048, 512
NTOK = BS * L              # 32768 tokens total
TOK = NTOK // NCORES       # 4096 tokens per core
INNER = 1024
CODES = 320
G = 2
VD = 128                   # codebook row dim
CHUNK = 512                # tokens per pipeline chunk
NCHUNK = TOK // CHUNK      # 8
KT1 = DIM // 128           # 4  k-tiles for mm1
IT = INNER // 128          # 8  inner tiles
TT = CHUNK // 128          # 4  token sub-tiles per chunk
NWARM = 8                  # PE p-state warmup matmuls
DEPTH = 2                  # chunks of software-pipeline lag between mm1/mm2

_CACHE = {}


def _round_f32r(a: np.ndarray) -> np.ndarray:
    """Round fp32 values to the f32r grid (drop 12 mantissa bits, RNE)."""
    u = np.ascontiguousarray(a, np.float32).view(np.uint32).copy()
    low = u & 0xFFF
    keep = u & np.uint32(0xFFFFF000)
    round_up = (low > 0x800) | ((low == 0x800) & (((u >> 12) & 1) == 1))
    keep = keep + (round_up.astype(np.uint32) << 12)
    return keep.view(np.float32)


def _build_nc():
    import concourse.bass as bass
    import concourse.tile as tile
    from concourse import bacc, mybir

    f32 = mybir.dt.float32
    f32r = mybir.dt.float32r
    u32 = mybir.dt.uint32
    ADD = mybir.AluOpType.add
    GELU = mybir.ActivationFunctionType.Gelu

    nc = bacc.Bacc("TRN2")
    xT = nc.dram_tensor("xT", [DIM, TOK], f32r, kind="ExternalInput")
    gum = nc.dram_tensor("gum", [TOK * G, CODES], f32, kind="ExternalInput")
    W1 = nc.dram_tensor("W1", [DIM, INNER], f32r, kind="ExternalInput")
    W2 = nc.dram_tensor("W2", [INNER, G * CODES], f32r, kind="ExternalInput")
    b1 = nc.dram_tensor("b1", [INNER], f32, kind="ExternalInput")
    emb = nc.dram_tensor("emb", [CODES, VD], f32, kind="ExternalInput")
    out = nc.dram_tensor("out", [TOK, G * VD], f32, kind="ExternalOutput")

    with tile.TileContext(nc) as tc:
        with (
            tc.tile_pool(name="consts", bufs=1) as consts,
            tc.tile_pool(name="xp", bufs=3) as xp,
            tc.tile_pool(name="hp", bufs=DEPTH + 1) as hp,
            tc.tile_pool(name="gp", bufs=3) as gp,
            tc.tile_pool(name="zp", bufs=3) as zp,
            tc.tile_pool(name="m8p", bufs=4) as m8p,
            tc.tile_pool(name="mip", bufs=2) as mip,
            tc.tile_pool(name="op", bufs=2) as op,
            tc.tile_pool(name="ps1", bufs=3, space="PSUM") as ps1,
            tc.tile_pool(name="ps2", bufs=2, space="PSUM") as ps2,
        ):
            warm = consts.tile([128, CHUNK], f32r)
            # W1 as one tile per 128-column block: mm1's i-th accumulation
            # only waits for block i's DMA, not the whole 2MB of W1
            w1sb = [consts.tile([128, KT1, 128], f32r, name=f"w1b{i}")
                    for i in range(IT)]
            w2sb = consts.tile([128, IT, G * CODES], f32r)
            b1sb = consts.tile([128, IT], f32)

            xTr = xT.rearrange("(k p) t -> p k t", p=128)
            W1r = W1.rearrange("(k p) i -> p k i", p=128)
            W2r = W2.rearrange("(k p) c -> p k c", p=128)
            # gumbels: row 2*tok+g -> [chunk, part(token), t, g, code]
            gumr = gum.rearrange("(c t p g) x -> c p t g x",
                                 t=TT, p=128, g=G)
            outr = out.rearrange("(c t p) x -> c p t x", t=TT, p=128)

            xsb = {}
            gsb = {}

            def issue_x(c, eng):
                t = xp.tile([128, KT1, CHUNK], f32r)
                xsb[c] = t
                eng.dma_start(t[:], xTr[:, :, c * CHUNK:(c + 1) * CHUNK])

            def issue_g(c, eng):
                t = gp.tile([128, TT, G, CODES], f32)
                gsb[c] = t
                eng.dma_start(t[:], gumr[c])

            # ---- prologue ------------------------------------------------
            nc.gpsimd.memset(warm[:].bitcast(u32), 0)
            # W1 blocks first on Act (mm1's critical path), x on SP,
            # W2 + first gumbels on the SWDGE queue (idle until gathers start)
            issue_x(0, nc.sync)
            for i in range(IT):
                nc.scalar.dma_start(w1sb[i][:],
                                    W1r[:, :, i * 128:(i + 1) * 128])
            nc.scalar.dma_start(b1sb[:], b1.rearrange("(i p) -> p i", p=128))
            issue_x(1, nc.sync)
            nc.gpsimd.dma_start(w2sb[:, :, 0:CODES], W2r[:, :, 0:CODES])
            issue_g(0, nc.gpsimd)
            nc.gpsimd.dma_start(w2sb[:, :, CODES:G * CODES],
                                W2r[:, :, CODES:G * CODES])

            # warmup matmuls: ramp the PE p-state while the prologue DMAs run
            for _ in range(NWARM):
                ph = ps1.tile([128, CHUNK], f32)
                nc.tensor.matmul(ph[:], warm[:, 0:128], warm[:],
                                 start=True, stop=True)

            hsb = {}
            for c in range(NCHUNK + DEPTH):
                # prefetch future chunks' inputs (both on the SP queue; the
                # Act queue must stay clear for gelus at startup)
                if 1 <= c + 1 < NCHUNK:
                    issue_g(c + 1, nc.sync)
                if c + 2 < NCHUNK:
                    issue_x(c + 2, nc.sync)

                if c < NCHUNK:
                    xs = xsb.pop(c)
                    hs = hp.tile([128, IT, CHUNK], f32r)
                    hsb[c] = hs
                    for i in range(IT):
                        ph = ps1.tile([128, CHUNK], f32)
                        for k in range(KT1):
                            nc.tensor.matmul(
                                ph[:],
                                w1sb[i][:, k, :],
                                xs[:, k, :],
                                start=(k == 0),
                                stop=(k == KT1 - 1),
                            )
                        nc.scalar.activation(hs[:, i, :], ph[:], GELU,
                                             bias=b1sb[:, i:i + 1])

                if c >= DEPTH:
                    cc = c - DEPTH
                    hs2 = hsb.pop(cc)
                    gs = gsb.pop(cc)
                    osb = op.tile([128, TT * G, VD], f32)
                    mi = mip.tile([128, TT * G, 8], u32)
                    for t in range(TT):
                        pz = ps2.tile([128, G, 512], f32)
                        for g2 in range(G):
                            for k in range(IT):
                                nc.tensor.matmul(
                                    pz[:, g2, 0:CODES],
                                    hs2[:, k, t * 128:(t + 1) * 128],
                                    w2sb[:, k, g2 * CODES:(g2 + 1) * CODES],
                                    start=(k == 0),
                                    stop=(k == IT - 1),
                                )
                        zsb = zp.tile([128, G, CODES], f32)
                        nc.vector.tensor_tensor(zsb[:], pz[:, :, 0:CODES],
                                                gs[:, t], op=ADD)
                        for g2 in range(G):
                            m8 = m8p.tile([128, 8], f32)
                            nc.vector.max(m8[:], zsb[:, g2, :])
                            nc.vector.max_index(mi[:, t * G + g2, :], m8[:],
                                                zsb[:, g2, :])
                            nc.gpsimd.indirect_dma_start(
                                out=osb[:, t * G + g2, :],
                                out_offset=None,
                                in_=emb[:],
                                in_offset=bass.IndirectOffsetOnAxis(
                                    ap=mi[:, t * G + g2, 0:1], axis=0),
                            )
                    nc.scalar.dma_start(outr[cc], osb[:])

    nc.compile()
    return nc


def kernel(**inputs) -> np.ndarray:
    from concourse.bass_utils import run_bass_kernel_spmd

    x = np.asarray(inputs["x"], np.float32)
    gumbels = np.asarray(inputs["gumbels"], np.float32)
    W1 = np.asarray(inputs["W1"], np.float32)
    b1 = np.asarray(inputs["b1"], np.float32)
    W2 = np.asarray(inputs["W2"], np.float32)
    b2 = np.asarray(inputs["b2"], np.float32)
    emb = np.asarray(inputs["emb"], np.float32)

    if "nc" not in _CACHE:
        _CACHE["nc"] = _build_nc()
    nc = _CACHE["nc"]

    xt = x.reshape(NTOK, DIM)
    W1r = _round_f32r(W1)
    W2r = _round_f32r(W2)
    # fold b2 into the gumbel noise: z = logits + b2 + gumbels
    gumb = gumbels.reshape(NTOK, G, CODES) + b2.reshape(G, CODES)
    gumb = gumb.reshape(NTOK * G, CODES)

    in_maps = []
    for c in range(NCORES):
        xT_c = _round_f32r(np.ascontiguousarray(xt[c * TOK:(c + 1) * TOK, :].T))
        in_maps.append({
            "xT": xT_c,
            "gum": np.ascontiguousarray(gumb[c * TOK * G:(c + 1) * TOK * G]),
            "W1": W1r,
            "W2": W2r,
            "b1": b1,
            "emb": emb,
        })

    trace = bool(int(os.environ.get("KERNEL_TRACE", "0")))
    res = run_bass_kernel_spmd(nc, in_maps, core_ids=list(range(NCORES)),
                               trace=trace)
    _CACHE["last_result"] = res
    outs = [res.results[c]["out"] for c in range(NCORES)]
    return np.concatenate(outs, axis=0).reshape(BS, L, G * VD)
